# revision 6
# baseline (speedup 1.0000x reference)
"""BASE-layer MoE kernel v2 for Trainium2, expert-parallel across 8 NeuronCores.

Changes vs v1 (504 us):
  - Residual stream kept transposed [d, c] end-to-end: host ships x^T and the
    layer-0 LN output h0^T pre-tiled, ff2 is computed in [d, c] orientation,
    so the 128 PE transposes (~35 us) disappear. Output y^T is transposed
    back on the host.
  - LN stats for layer 1 are computed on PE with ones-matmuls over the
    residual (fp32r moving operand = 1 cycle/row), finalized on [1, C]
    vectors, broadcast back to [128, C] via K=1 matmuls.
  - A tuned fraction of the contraction runs in fp8 e4m3 with
    perf_mode=DoubleRow (2 K-tiles per matmul): the last 2*P1 of 8 d-tiles in
    ff1 and the last 2*P2 of 32 f-tiles in ff2. Config (P1=1, P2=2) measures
    1.77e-2 rel err in simulation vs the 2e-2 budget (bf16-only: 2.5e-3).
  - Everything lives in a power-of-2 scaled domain so bf16/fp8 parts share
    one PSUM accumulation chain: residual x~ = kappa*x with
    kappa = S_H1*S_W2; LN is scale-invariant once eps is scaled by kappa^2.
"""

import contextlib

import numpy as np

import concourse.bass as bass
import concourse.mybir as mybir
import concourse.tile as tile
from concourse.bass_utils import run_bass_kernel_spmd

S, B, D, F, E, L = 2048, 4, 1024, 4096, 8, 2
EPS = 1e-5
T = S * B
C = T // E
P = 128
CH, CW = 2, 512          # c halves
DT = D // P              # 8 d tiles
FT = F // P              # 32 f tiles

# fp8 DoubleRow config: last 2*P1 d-tiles of ff1 / last 2*P2 f-tiles of ff2
P1, P2 = 1, 2
S_H, S_W1, S_H1, S_W2 = 16.0, 64.0, 32.0, 32.0
KAPPA = S_H1 * S_W2                      # residual scale (1024, power of 2)
SIG1 = S_H1 / (S_H * S_W1)               # ff1 psum -> h1 rescale
EPS_SQ = EPS * KAPPA * KAPPA / (S_H * S_H)   # bias for the sqrt in 1/A
RD = 1.0 / D

F32 = mybir.dt.float32
F32R = mybir.dt.float32r
BF16 = mybir.dt.bfloat16
FP8 = mybir.dt.float8e4
DR = mybir.MatmulPerfMode.DoubleRowSwInterleave

_MAX_WAITS = 1
_NAME_CTR = [0]


def _nm(base):
    _NAME_CTR[0] += 1
    return f"{base}_{_NAME_CTR[0]}"


def _split_multi_waits(nc, limit=_MAX_WAITS):
    """walrus build rejects >1 sync wait per instruction; split onto NOPs."""
    n_split = 0
    for f in nc.m.functions:
        for bb in f.blocks:
            out = []
            changed = False
            for ins in bb.instructions:
                si = getattr(ins, "sync_info", None)
                if si is not None and si.on_wait and len(si.on_wait) > limit:
                    waits = list(si.on_wait)
                    head, tail = waits[:-limit], waits[-limit:]
                    for i in range(0, len(head), limit):
                        n_split += 1
                        nop = mybir.InstNoOp(
                            name=f"waitsplit-{n_split}",
                            engine=ins.engine,
                            text_hint="waitsplit",
                            bass_nofuse=True,
                        )
                        nop.sync_info = mybir.SyncInfo(
                            on_wait=head[i : i + limit], on_update=[]
                        )
                        out.append(nop)
                    ins.sync_info = mybir.SyncInfo(
                        on_wait=tail, on_update=list(si.on_update or [])
                    )
                    changed = True
                out.append(ins)
            if changed:
                bb.instructions = out
    return n_split


def build_bass(p1=P1, p2=P2, with_b2=False, split_waits=True):
    kb1 = DT - 2 * p1
    kb2 = FT - 2 * p2
    nc = bass.Bass()
    x_d = nc.declare_dram_parameter("x", [D, C], F32, isOutput=False)
    ht0b_d = nc.declare_dram_parameter("ht0b", [P, kb1, C], BF16, isOutput=False)
    if p1:
        ht0f_d = nc.declare_dram_parameter("ht0f", [P, 2 * p1, C], FP8, isOutput=False)
    w1b_d = nc.declare_dram_parameter(
        "w1b", [L, FT // 2, P, 2, kb1, P], BF16, isOutput=False
    )
    if p1:
        w1f_d = nc.declare_dram_parameter(
            "w1f", [L, FT // 2, P, 2, 2 * p1, P], FP8, isOutput=False
        )
    w2b_d = nc.declare_dram_parameter("w2b", [L, DT, P, kb2, P], BF16, isOutput=False)
    if p2:
        w2f_d = nc.declare_dram_parameter(
            "w2f", [L, DT, P, p2, 2, P], FP8, isOutput=False
        )
    b1_d = nc.declare_dram_parameter("b1", [L, P, FT], F32, isOutput=False)
    alpha_d = nc.declare_dram_parameter("alpha", [P, C], F32, isOutput=False)
    if with_b2:
        b2_d = nc.declare_dram_parameter("b2", [L, P, DT], F32, isOutput=False)
    y_d = nc.declare_dram_parameter("y", [D, C], F32, isOutput=True)

    with tile.TileContext(nc) as tc:
        with contextlib.ExitStack() as ctx:
            singles = ctx.enter_context(tc.tile_pool(name="singles", bufs=1))
            xspool = ctx.enter_context(tc.tile_pool(name="xspool", bufs=1))
            htpool = ctx.enter_context(tc.tile_pool(name="htpool", bufs=1))
            h1pool = ctx.enter_context(tc.tile_pool(name="h1pool", bufs=1))
            w1pool = ctx.enter_context(tc.tile_pool(name="w1pool", bufs=8))
            w2pool = ctx.enter_context(tc.tile_pool(name="w2pool", bufs=3))
            stat = ctx.enter_context(tc.tile_pool(name="stat", bufs=2))
            tmps = ctx.enter_context(tc.tile_pool(name="tmps", bufs=3))
            abpool = ctx.enter_context(tc.tile_pool(name="abpool", bufs=1))
            ps1 = ctx.enter_context(tc.tile_pool(name="ps1", bufs=3, space="PSUM"))
            ps2 = ctx.enter_context(tc.tile_pool(name="ps2", bufs=2, space="PSUM"))
            psm = ctx.enter_context(tc.tile_pool(name="psm", bufs=1, space="PSUM"))

            # ---- layer-0 h^T (host-precomputed LN output), ch0 first ----
            # per-(ch, kt) tiles: fine DMA granularity so the first ff1 chain
            # starts as soon as the first 256 KB lands
            htb = {
                (ch, kt): htpool.tile(
                    [P, CW], BF16, tag=f"htb{ch}_{kt}", name=_nm(f"htb{ch}_{kt}")
                )
                for ch in range(CH)
                for kt in range(kb1)
            }
            htf = (
                [htpool.tile([P, 2 * p1, CW], FP8, tag=f"htf{ch}", name=_nm(f"htf{ch}")) for ch in range(CH)]
                if p1
                else None
            )
            if p1:
                for ch in range(CH):
                    csl = slice(ch * CW, (ch + 1) * CW)
                    eng = nc.gpsimd if ch == 0 else nc.scalar
                    eng.dma_start(out=htf[ch], in_=ht0f_d[:, :, csl])
            for kt in range(kb1):
                for ch in range(CH):
                    csl = slice(ch * CW, (ch + 1) * CW)
                    eng = nc.gpsimd if ch == 0 else nc.scalar
                    eng.dma_start(out=htb[(ch, kt)], in_=ht0b_d[:, kt, csl])

            # ---- small consts + residual x~ ----
            b1_sb = singles.tile([P, L, FT], F32)
            for l in range(L):
                nc.scalar.dma_start(out=b1_sb[:, l, :], in_=b1_d[l])
            if with_b2:
                b2_sb = singles.tile([P, L, DT], F32)
                for l in range(L):
                    nc.scalar.dma_start(out=b2_sb[:, l, :], in_=b2_d[l])
            xs = {}
            for dt in range(DT):
                for ch in range(CH):
                    t = xspool.tile([P, CW], F32, tag=f"xs{dt}_{ch}", name=_nm(f"xs{dt}_{ch}"))
                    nc.gpsimd.dma_start(
                        out=t,
                        in_=x_d[dt * P : (dt + 1) * P, ch * CW : (ch + 1) * CW],
                    )
                    xs[(dt, ch)] = t
            alpha_sb = singles.tile([P, C], F32)

            onesb = singles.tile([P, 1], BF16)
            nc.gpsimd.memset(onesb, RD)
            ones1 = singles.tile([1, P], BF16)
            nc.gpsimd.memset(ones1, 1.0)
            eps_t = singles.tile([1, 1], F32)
            nc.gpsimd.memset(eps_t, EPS_SQ)

            h1b = [h1pool.tile([P, kb2, CW], BF16, tag=f"h1b{ch}", name=_nm(f"h1b{ch}")) for ch in range(CH)]
            h1f = (
                [h1pool.tile([P, 2 * p2, CW], FP8, tag=f"h1f{ch}", name=_nm(f"h1f{ch}")) for ch in range(CH)]
                if p2
                else None
            )

            # ---------------- emitters ----------------
            def emit_ff1(l, chs, inject=None):
                """ff1 pass. With two c-halves the chains are interleaved so
                consecutive matmuls share the stationary operand (one weight
                load serves both)."""
                for fti in range(FT // 2):
                    if p1:
                        w1ft = w1pool.tile([P, 2, 2 * p1, P], FP8, tag="w1f", name=_nm("w1f"))
                        nc.sync.dma_start(out=w1ft, in_=w1f_d[l, fti])
                    w1bt = w1pool.tile([P, 2, kb1, P], BF16, tag="w1b", name=_nm("w1b"))
                    nc.sync.dma_start(out=w1bt, in_=w1b_d[l, fti])
                    for fi in range(2):
                        ft = 2 * fti + fi
                        pts = {
                            ch: ps1.tile([P, CW], F32, tag="ps1", name=_nm("ps1"))
                            for ch in chs
                        }
                        if p1:
                            for ch in chs:
                                nc.tensor.matmul(
                                    pts[ch],
                                    lhsT=w1ft[:, fi, :, :],
                                    rhs=htf[ch][:, :, :],
                                    start=True,
                                    stop=False,
                                    perf_mode=DR,
                                )
                        for kt in range(kb1):
                            for ch in chs:
                                nc.tensor.matmul(
                                    pts[ch],
                                    lhsT=w1bt[:, fi, kt, :],
                                    rhs=htb[(ch, kt)][:, :],
                                    start=(kt == 0 and not p1),
                                    stop=(kt == kb1 - 1),
                                )
                        for ch in chs:
                            out_ap = (
                                h1b[ch][:, ft, :]
                                if ft < kb2
                                else h1f[ch][:, ft - kb2, :]
                            )
                            nc.scalar.activation(
                                out=out_ap,
                                in_=pts[ch],
                                func=mybir.ActivationFunctionType.Relu,
                                bias=b1_sb[:, l, ft : ft + 1],
                                scale=SIG1,
                            )
                    if inject and fti in inject:
                        for fn in inject[fti]:
                            fn()

            sqs = {}
            xbs = {}

            def emit_stats_mm(l, ch, dt, smu_t, sm2_t):
                nc.tensor.matmul(
                    smu_t,
                    lhsT=onesb[:, :],
                    rhs=xbs[(dt, ch)][:, :],
                    start=(dt == 0),
                    stop=(dt == DT - 1),
                )
                nc.tensor.matmul(
                    sm2_t,
                    lhsT=onesb[:, :],
                    rhs=sqs[(dt, ch)][:, :],
                    start=(dt == 0),
                    stop=(dt == DT - 1),
                )

            def emit_ff2(l, ch, inject=None):
                """ff2 pass over dt; returns carry closure (last stats MM)."""
                last = l == L - 1
                smu_t = sm2_t = None
                if not last:
                    smu_t = psm.tile([1, CW], F32, tag="smu", name=_nm("smu"))
                    sm2_t = psm.tile([1, CW], F32, tag="sm2", name=_nm("sm2"))
                carry = None
                for dt in range(DT):
                    w2bt = w2pool.tile([P, kb2, P], BF16, tag="w2b", name=_nm("w2b"))
                    nc.gpsimd.dma_start(out=w2bt, in_=w2b_d[l, dt])
                    if p2:
                        w2ft = w2pool.tile([P, p2, 2, P], FP8, tag="w2f", name=_nm("w2f"))
                        nc.gpsimd.dma_start(out=w2ft, in_=w2f_d[l, dt])
                    pt = ps2.tile([P, CW], F32, tag="ps2", name=_nm("ps2"))
                    for kt in range(kb2):
                        nc.tensor.matmul(
                            pt,
                            lhsT=w2bt[:, kt, :],
                            rhs=h1b[ch][:, kt, :],
                            start=(kt == 0),
                            stop=(kt == kb2 - 1 and not p2),
                        )
                    for a in range(p2):
                        nc.tensor.matmul(
                            pt,
                            lhsT=w2ft[:, a, :, :],
                            rhs=h1f[ch][:, 2 * a : 2 * a + 2, :],
                            start=False,
                            stop=(a == p2 - 1),
                            perf_mode=DR,
                        )
                    x = xs[(dt, ch)]
                    if with_b2:
                        tmp = tmps.tile([P, CW], F32, tag="evt", name=_nm("evt"))
                        nc.scalar.activation(
                            out=tmp,
                            in_=pt,
                            func=mybir.ActivationFunctionType.Identity,
                            bias=b2_sb[:, l, dt : dt + 1],
                            scale=1.0,
                        )
                        nc.vector.tensor_add(out=x, in0=x, in1=tmp)
                    else:
                        nc.vector.tensor_add(out=x, in0=x, in1=pt)
                    if last:
                        nc.vector.tensor_mul(
                            out=x, in0=x, in1=alpha_sb[:, ch * CW : (ch + 1) * CW]
                        )
                        nc.scalar.dma_start(
                            out=y_d[dt * P : (dt + 1) * P, ch * CW : (ch + 1) * CW],
                            in_=x,
                        )
                    else:
                        xb = tmps.tile([P, CW], BF16, tag="xb", name=_nm("xb"))
                        nc.vector.tensor_copy(out=xb, in_=x)
                        xbs[(dt, ch)] = xb
                        sq = tmps.tile([P, CW], BF16, tag="sq", name=_nm("sq"))
                        nc.vector.tensor_mul(out=sq, in0=xb, in1=xb)
                        sqs[(dt, ch)] = sq
                        # delay-slot: stats MM for dt-1 lands after chain dt
                        if dt > 0:
                            emit_stats_mm(l, ch, dt - 1, smu_t, sm2_t)
                    if inject and dt in inject:
                        for fn in inject[dt]:
                            fn()
                if not last:
                    carry = lambda: emit_stats_mm(l, ch, DT - 1, smu_t, sm2_t)
                return carry, smu_t, sm2_t

            def emit_ff2_pair(l):
                """Last-layer ff2: both c-halves interleaved per dt, sharing
                one w2 load; evict + gate + output DMA per (dt, ch)."""
                for dt in range(DT):
                    w2bt = w2pool.tile([P, kb2, P], BF16, tag="w2b", name=_nm("w2b"))
                    nc.gpsimd.dma_start(out=w2bt, in_=w2b_d[l, dt])
                    if p2:
                        w2ft = w2pool.tile([P, p2, 2, P], FP8, tag="w2f", name=_nm("w2f"))
                        nc.gpsimd.dma_start(out=w2ft, in_=w2f_d[l, dt])
                    pts = {
                        ch: ps2.tile([P, CW], F32, tag="ps2", name=_nm("ps2"))
                        for ch in range(CH)
                    }
                    for kt in range(kb2):
                        for ch in range(CH):
                            nc.tensor.matmul(
                                pts[ch],
                                lhsT=w2bt[:, kt, :],
                                rhs=h1b[ch][:, kt, :],
                                start=(kt == 0),
                                stop=(kt == kb2 - 1 and not p2),
                            )
                    for a in range(p2):
                        for ch in range(CH):
                            nc.tensor.matmul(
                                pts[ch],
                                lhsT=w2ft[:, a, :, :],
                                rhs=h1f[ch][:, 2 * a : 2 * a + 2, :],
                                start=False,
                                stop=(a == p2 - 1),
                                perf_mode=DR,
                            )
                    for ch in range(CH):
                        x = xs[(dt, ch)]
                        if with_b2:
                            tmp = tmps.tile([P, CW], F32, tag="evt", name=_nm("evt"))
                            nc.scalar.activation(
                                out=tmp,
                                in_=pts[ch],
                                func=mybir.ActivationFunctionType.Identity,
                                bias=b2_sb[:, l, dt : dt + 1],
                                scale=1.0,
                            )
                            nc.vector.tensor_add(out=x, in0=x, in1=tmp)
                        else:
                            nc.vector.tensor_add(out=x, in0=x, in1=pts[ch])
                        nc.vector.tensor_mul(
                            out=x, in0=x, in1=alpha_sb[:, ch * CW : (ch + 1) * CW]
                        )
                        nc.scalar.dma_start(
                            out=y_d[dt * P : (dt + 1) * P, ch * CW : (ch + 1) * CW],
                            in_=x,
                        )

            ab_sb = {}

            def make_finalize(l, ch, smu_t, sm2_t):
                def fin():
                    mu_sb = stat.tile([1, CW], F32, tag="mu_sb", name=_nm("mu_sb"))
                    nc.vector.tensor_copy(out=mu_sb, in_=smu_t)
                    musq = stat.tile([1, CW], F32, tag="musq", name=_nm("musq"))
                    nc.vector.tensor_mul(out=musq, in0=mu_sb, in1=mu_sb)
                    var = stat.tile([1, CW], F32, tag="var", name=_nm("var"))
                    nc.vector.tensor_sub(out=var, in0=sm2_t, in1=musq)
                    sd = stat.tile([1, CW], F32, tag="sd", name=_nm("sd"))
                    nc.scalar.activation(
                        out=sd,
                        in_=var,
                        func=mybir.ActivationFunctionType.Sqrt,
                        bias=eps_t,
                        scale=1.0 / (S_H * S_H),
                    )
                    A = stat.tile([1, CW], BF16, tag="A", name=_nm("A"))
                    muA = stat.tile([1, CW], BF16, tag="muA", name=_nm("muA"))
                    with nc.allow_low_precision(reason="rstd/mu*rstd in bf16 feed bf16 matmuls anyway"):
                        nc.vector.reciprocal(out=A, in_=sd)
                        nc.vector.tensor_mul(out=muA, in0=mu_sb, in1=A)
                    ab_sb[("A", ch)] = A
                    ab_sb[("muA", ch)] = muA

                return fin

            def make_bcast(l, ch):
                def bc():
                    for name in ("A", "muA"):
                        bct = psm.tile([P, CW], F32, tag="bc", name=_nm("bc"))
                        nc.tensor.matmul(
                            bct,
                            lhsT=ones1[:, :],
                            rhs=ab_sb[(name, ch)][:, :],
                            start=True,
                            stop=True,
                        )
                        sb = abpool.tile([P, CW], F32, tag=f"{name}b{ch}", name=_nm(f"{name}b{ch}"))
                        nc.vector.tensor_copy(out=sb, in_=bct)
                        ab_sb[(name + "b", ch)] = sb

                return bc

            def make_apply(l, ch):
                def ap():
                    Ab = ab_sb[("Ab", ch)]
                    mAb = ab_sb[("muAb", ch)]
                    for dt in range(DT):
                        tmp = tmps.tile([P, CW], F32, tag="tap", name=_nm("tap"))
                        nc.vector.tensor_mul(out=tmp, in0=xs[(dt, ch)], in1=Ab)
                        out_ap = (
                            htb[(ch, dt)][:, :]
                            if dt < kb1
                            else htf[ch][:, dt - kb1, :]
                        )
                        nc.vector.tensor_sub(out=out_ap, in0=tmp, in1=mAb)

                return ap

            # ---------------- program ----------------
            emit_ff1(0, [0, 1])
            carry0, smu0, sm20 = emit_ff2(0, 0)
            fin0 = make_finalize(0, 0, smu0, sm20)
            carry1, smu1, sm21 = emit_ff2(
                0,
                1,
                inject={
                    0: [carry0],
                    1: [fin0],
                    2: [make_bcast(0, 0)],
                    3: [make_apply(0, 0)],
                },
            )
            fin1 = make_finalize(0, 1, smu1, sm21)
            emit_ff1(
                1,
                [0],
                inject={
                    0: [carry1],
                    1: [fin1],
                    2: [make_bcast(0, 1)],
                    3: [make_apply(0, 1)],
                },
            )
            emit_ff1(1, [1])
            nc.sync.dma_start(out=alpha_sb, in_=alpha_d[:, :])
            emit_ff2(1, 0)
            emit_ff2(1, 1)

    if split_waits:
        _split_multi_waits(nc)
    return nc


_NC_CACHE = {}


def _get_nc(with_b2):
    key = (P1, P2, with_b2)
    if key not in _NC_CACHE:
        _NC_CACHE[key] = build_bass(P1, P2, with_b2)
    return _NC_CACHE[key]


# ---------------------------------------------------------------------------
# Host side
# ---------------------------------------------------------------------------
def _routing_perm(features, centroids):
    """Replicates reference._balanced_assignment bit-for-bit on CPU jax."""
    import jax
    import jax.numpy as jnp

    with jax.default_device(jax.devices("cpu")[0]):
        feats = jnp.asarray(features)
        cents = jnp.asarray(centroids)
        aff = jax.lax.stop_gradient(feats) @ jax.lax.stop_gradient(cents).T
        aff = jnp.nan_to_num(aff)
        capacity = feats.shape[0] // cents.shape[0]
        order = jnp.argsort(-aff.max(axis=1))
        aff_ord = aff[order]

        def step(counts, row):
            masked = jnp.where(counts < capacity, row, -jnp.inf)
            e = jnp.argmax(masked).astype(jnp.int32)
            return counts.at[e].add(1), e

        _, assign_ord = jax.lax.scan(
            step, jnp.zeros(cents.shape[0], jnp.int32), aff_ord
        )
        assign = jnp.zeros(feats.shape[0], jnp.int32).at[order].set(assign_ord)
        return np.asarray(jnp.argsort(assign))


def _q8(x, scale):
    import ml_dtypes

    return np.clip(x * scale, -240.0, 240.0).astype(ml_dtypes.float8_e4m3)


def _swi(wpair):
    """[..., 2, M] pair -> DoubleRowSwInterleave layout [..., 2, M]:
    stream A[M-1], B[M-1], A[M-2], ... per partition row."""
    a = wpair[..., 0, ::-1]
    b = wpair[..., 1, ::-1]
    out = np.empty_like(wpair).reshape(*wpair.shape[:-2], 2 * wpair.shape[-1])
    out[..., 0::2] = a
    out[..., 1::2] = b
    return out.reshape(wpair.shape)


def _prep_core_inputs(xr, centroids, ln_gamma, ln_beta, W1, b1, W2, b2):
    import ml_dtypes

    kb1 = DT - 2 * P1
    kb2 = FT - 2 * P2
    with_b2 = bool(np.any(b2))
    maps = []
    for e in range(E):
        x = xr[e].astype(np.float32)  # [C, D]
        m = {}
        # alpha gate (computed from unscaled x), shipped /kappa, broadcast
        aff = x @ centroids[e].astype(np.float32)
        alpha = 1.0 / (1.0 + np.exp(-aff)) / KAPPA
        m["alpha"] = np.ascontiguousarray(
            np.broadcast_to(alpha[None, :].astype(np.float32), (P, C))
        )
        # layer-0 LN on host
        mu = x.mean(-1, keepdims=True)
        var = ((x - mu) ** 2).mean(-1, keepdims=True)
        h0 = (x - mu) / np.sqrt(var + EPS)  # [C, D]
        h0t = np.ascontiguousarray(h0.T * S_H).reshape(DT, P, C)  # [dt, p, c]
        m["ht0b"] = np.ascontiguousarray(
            h0t[:kb1].transpose(1, 0, 2)
        ).astype(ml_dtypes.bfloat16)
        if P1:
            m["ht0f"] = np.ascontiguousarray(
                np.clip(h0t[kb1:], -240.0, 240.0).transpose(1, 0, 2)
            ).astype(ml_dtypes.float8_e4m3)
        m["x"] = np.ascontiguousarray(x.T * KAPPA).astype(np.float32)

        w1b = np.empty((L, FT // 2, P, 2, kb1, P), ml_dtypes.bfloat16)
        w1f = np.empty((L, FT // 2, P, 2, 2 * P1, P), ml_dtypes.float8_e4m3)
        w2b = np.empty((L, DT, P, kb2, P), ml_dtypes.bfloat16)
        w2f = np.empty((L, DT, P, P2, 2, P), ml_dtypes.float8_e4m3)
        b1s = np.empty((L, P, FT), np.float32)
        b2s = np.empty((L, P, DT), np.float32)
        for l in range(L):
            g = ln_gamma[l, e].astype(np.float32)
            bt = ln_beta[l, e].astype(np.float32)
            w1_eff = W1[l, e].astype(np.float32) * g[None, :]  # [F, D]
            b1_eff = (b1[l, e] + W1[l, e] @ bt).astype(np.float32) * S_H1
            # [fti, fi, m, kt, p] -> [fti, p, fi, kt, m]
            a = w1_eff.reshape(FT // 2, 2, P, DT, P).transpose(0, 4, 1, 3, 2)
            w1b[l] = (a[:, :, :, :kb1, :] * S_W1).astype(ml_dtypes.bfloat16)
            if P1:
                w1f[l] = _swi(_q8(a[:, :, :, kb1:, :], S_W1))
            # W2 [D, F] -> [dt, m, kt, p] -> [dt, p, kt, m]
            a2 = W2[l, e].astype(np.float32).reshape(DT, P, FT, P).transpose(
                0, 3, 2, 1
            )
            w2b[l] = (a2[:, :, :kb2, :] * S_W2).astype(ml_dtypes.bfloat16)
            if P2:
                w2f[l] = _swi(
                    _q8(a2[:, :, kb2:, :].reshape(DT, P, P2, 2, P), S_W2)
                )
            b1s[l] = b1_eff.reshape(FT, P).T
            b2s[l] = (b2[l, e].astype(np.float32) * KAPPA).reshape(DT, P).T
        m["w1b"] = w1b
        m["w2b"] = w2b
        if P1:
            m["w1f"] = w1f
        if P2:
            m["w2f"] = w2f
        m["b1"] = b1s
        if with_b2:
            m["b2"] = b2s
        maps.append(m)
    return maps, with_b2


def kernel(
    input_features,
    centroids,
    ln_gamma,
    ln_beta,
    W1,
    b1,
    W2,
    b2,
    input_ids=None,
    _trace=False,
    _tmpdir=None,
):
    input_features = np.asarray(input_features, np.float32)
    centroids = np.asarray(centroids, np.float32)
    ln_gamma = np.asarray(ln_gamma, np.float32)
    ln_beta = np.asarray(ln_beta, np.float32)
    W1 = np.asarray(W1, np.float32)
    b1 = np.asarray(b1, np.float32)
    W2 = np.asarray(W2, np.float32)
    b2 = np.asarray(b2, np.float32)

    feats = input_features.reshape(T, D)
    perm = _routing_perm(feats, centroids)
    xr = feats[perm].reshape(E, C, D)

    maps, with_b2 = _prep_core_inputs(
        xr, centroids, ln_gamma, ln_beta, W1, b1, W2, b2
    )
    nc = _get_nc(with_b2)
    res = run_bass_kernel_spmd(
        nc, maps, list(range(E)), trace=_trace, tmpdir=_tmpdir
    )
    y = np.concatenate(
        [np.ascontiguousarray(res.results[e]["y"].T) for e in range(E)], axis=0
    )
    out = np.zeros((T, D), np.float32)
    out[perm] = y
    out = out.reshape(input_features.shape)
    if _trace:
        return out, res
    return out


# revision 7
# speedup vs baseline: 1.0036x; 1.0036x over previous
"""BASE-layer MoE kernel v2 for Trainium2, expert-parallel across 8 NeuronCores.

Changes vs v1 (504 us):
  - Residual stream kept transposed [d, c] end-to-end: host ships x^T and the
    layer-0 LN output h0^T pre-tiled, ff2 is computed in [d, c] orientation,
    so the 128 PE transposes (~35 us) disappear. Output y^T is transposed
    back on the host.
  - LN stats for layer 1 are computed on PE with ones-matmuls over the
    residual (fp32r moving operand = 1 cycle/row), finalized on [1, C]
    vectors, broadcast back to [128, C] via K=1 matmuls.
  - A tuned fraction of the contraction runs in fp8 e4m3 with
    perf_mode=DoubleRow (2 K-tiles per matmul): the last 2*P1 of 8 d-tiles in
    ff1 and the last 2*P2 of 32 f-tiles in ff2. Config (P1=1, P2=2) measures
    1.77e-2 rel err in simulation vs the 2e-2 budget (bf16-only: 2.5e-3).
  - Everything lives in a power-of-2 scaled domain so bf16/fp8 parts share
    one PSUM accumulation chain: residual x~ = kappa*x with
    kappa = S_H1*S_W2; LN is scale-invariant once eps is scaled by kappa^2.
"""

import contextlib

import numpy as np

import concourse.bass as bass
import concourse.mybir as mybir
import concourse.tile as tile
from concourse.bass_utils import run_bass_kernel_spmd

S, B, D, F, E, L = 2048, 4, 1024, 4096, 8, 2
EPS = 1e-5
T = S * B
C = T // E
P = 128
CH, CW = 2, 512          # c halves
DT = D // P              # 8 d tiles
FT = F // P              # 32 f tiles

# fp8 DoubleRow config: last 2*P1 d-tiles of ff1 / last 2*P2 f-tiles of ff2
P1, P2 = 1, 2
S_H, S_W1, S_H1, S_W2 = 16.0, 64.0, 32.0, 32.0
KAPPA = S_H1 * S_W2                      # residual scale (1024, power of 2)
SIG1 = S_H1 / (S_H * S_W1)               # ff1 psum -> h1 rescale
EPS_SQ = EPS * KAPPA * KAPPA / (S_H * S_H)   # bias for the sqrt in 1/A
RD = 1.0 / D

F32 = mybir.dt.float32
F32R = mybir.dt.float32r
BF16 = mybir.dt.bfloat16
FP8 = mybir.dt.float8e4
DR = mybir.MatmulPerfMode.DoubleRowSwInterleave

_MAX_WAITS = 1
_NAME_CTR = [0]


def _nm(base):
    _NAME_CTR[0] += 1
    return f"{base}_{_NAME_CTR[0]}"


def _split_multi_waits(nc, limit=_MAX_WAITS):
    """walrus build rejects >1 sync wait per instruction; split onto NOPs."""
    n_split = 0
    for f in nc.m.functions:
        for bb in f.blocks:
            out = []
            changed = False
            for ins in bb.instructions:
                si = getattr(ins, "sync_info", None)
                if si is not None and si.on_wait and len(si.on_wait) > limit:
                    waits = list(si.on_wait)
                    head, tail = waits[:-limit], waits[-limit:]
                    for i in range(0, len(head), limit):
                        n_split += 1
                        nop = mybir.InstNoOp(
                            name=f"waitsplit-{n_split}",
                            engine=ins.engine,
                            text_hint="waitsplit",
                            bass_nofuse=True,
                        )
                        nop.sync_info = mybir.SyncInfo(
                            on_wait=head[i : i + limit], on_update=[]
                        )
                        out.append(nop)
                    ins.sync_info = mybir.SyncInfo(
                        on_wait=tail, on_update=list(si.on_update or [])
                    )
                    changed = True
                out.append(ins)
            if changed:
                bb.instructions = out
    return n_split


def build_bass(p1=P1, p2=P2, with_b2=False, split_waits=True):
    kb1 = DT - 2 * p1
    kb2 = FT - 2 * p2
    nc = bass.Bass()
    x_d = nc.declare_dram_parameter("x", [D, C], F32, isOutput=False)
    ht0b_d = nc.declare_dram_parameter("ht0b", [P, kb1, C], BF16, isOutput=False)
    if p1:
        ht0f_d = nc.declare_dram_parameter("ht0f", [P, 2 * p1, C], FP8, isOutput=False)
    w1b_d = nc.declare_dram_parameter(
        "w1b", [L, FT // 2, P, 2, kb1, P], BF16, isOutput=False
    )
    if p1:
        w1f_d = nc.declare_dram_parameter(
            "w1f", [L, FT // 2, P, 2, 2 * p1, P], FP8, isOutput=False
        )
    w2b_d = nc.declare_dram_parameter("w2b", [L, DT, P, kb2, P], BF16, isOutput=False)
    if p2:
        w2f_d = nc.declare_dram_parameter(
            "w2f", [L, DT, P, p2, 2, P], FP8, isOutput=False
        )
    b1_d = nc.declare_dram_parameter("b1", [L, P, FT], F32, isOutput=False)
    alpha_d = nc.declare_dram_parameter("alpha", [P, C], F32, isOutput=False)
    if with_b2:
        b2_d = nc.declare_dram_parameter("b2", [L, P, DT], F32, isOutput=False)
    y_d = nc.declare_dram_parameter("y", [D, C], F32, isOutput=True)

    with tile.TileContext(nc) as tc:
        with contextlib.ExitStack() as ctx:
            singles = ctx.enter_context(tc.tile_pool(name="singles", bufs=1))
            xspool = ctx.enter_context(tc.tile_pool(name="xspool", bufs=1))
            htpool = ctx.enter_context(tc.tile_pool(name="htpool", bufs=1))
            h1pool = ctx.enter_context(tc.tile_pool(name="h1pool", bufs=1))
            w1pool = ctx.enter_context(tc.tile_pool(name="w1pool", bufs=8))
            w2pool = ctx.enter_context(tc.tile_pool(name="w2pool", bufs=3))
            stat = ctx.enter_context(tc.tile_pool(name="stat", bufs=2))
            tmps = ctx.enter_context(tc.tile_pool(name="tmps", bufs=3))
            abpool = ctx.enter_context(tc.tile_pool(name="abpool", bufs=1))
            ps1 = ctx.enter_context(tc.tile_pool(name="ps1", bufs=3, space="PSUM"))
            ps2 = ctx.enter_context(tc.tile_pool(name="ps2", bufs=2, space="PSUM"))
            psm = ctx.enter_context(tc.tile_pool(name="psm", bufs=1, space="PSUM"))

            # ---- PE warmup: release the HAM clock gate during startup DMAs ----
            warm = singles.tile([P, CW], BF16, name="warm")
            nc.vector.memset(warm, 0.0)
            wps = psm.tile([P, CW], F32, tag="bc", name=_nm("wps"))
            for _ in range(14):
                nc.tensor.matmul(
                    wps, lhsT=warm[:, 0:P], rhs=warm, start=True, stop=True
                )

            # ---- layer-0 h^T (host-precomputed LN output), ch0 first ----
            # per-(ch, kt) tiles: fine DMA granularity so the first ff1 chain
            # starts as soon as the first 256 KB lands
            htb = {
                (ch, kt): htpool.tile(
                    [P, CW], BF16, tag=f"htb{ch}_{kt}", name=_nm(f"htb{ch}_{kt}")
                )
                for ch in range(CH)
                for kt in range(kb1)
            }
            htf = (
                [htpool.tile([P, 2 * p1, CW], FP8, tag=f"htf{ch}", name=_nm(f"htf{ch}")) for ch in range(CH)]
                if p1
                else None
            )
            for ch in range(CH):
                csl = slice(ch * CW, (ch + 1) * CW)
                for kt in range(kb1):
                    eng = nc.gpsimd if kt % 2 == 0 else nc.scalar
                    eng.dma_start(out=htb[(ch, kt)], in_=ht0b_d[:, kt, csl])
                if p1:
                    nc.gpsimd.dma_start(out=htf[ch], in_=ht0f_d[:, :, csl])

            # ---- small consts + residual x~ ----
            b1_sb = singles.tile([P, L, FT], F32)
            for l in range(L):
                nc.scalar.dma_start(out=b1_sb[:, l, :], in_=b1_d[l])
            if with_b2:
                b2_sb = singles.tile([P, L, DT], F32)
                for l in range(L):
                    nc.scalar.dma_start(out=b2_sb[:, l, :], in_=b2_d[l])
            xs = {}
            for dt in range(DT):
                for ch in range(CH):
                    t = xspool.tile([P, CW], F32, tag=f"xs{dt}_{ch}", name=_nm(f"xs{dt}_{ch}"))
                    nc.gpsimd.dma_start(
                        out=t,
                        in_=x_d[dt * P : (dt + 1) * P, ch * CW : (ch + 1) * CW],
                    )
                    xs[(dt, ch)] = t
            alpha_sb = singles.tile([P, C], F32)

            onesb = singles.tile([P, 1], BF16)
            nc.gpsimd.memset(onesb, RD)
            ones1 = singles.tile([1, P], BF16)
            nc.gpsimd.memset(ones1, 1.0)
            eps_t = singles.tile([1, 1], F32)
            nc.gpsimd.memset(eps_t, EPS_SQ)

            h1b = [h1pool.tile([P, kb2, CW], BF16, tag=f"h1b{ch}", name=_nm(f"h1b{ch}")) for ch in range(CH)]
            h1f = (
                [h1pool.tile([P, 2 * p2, CW], FP8, tag=f"h1f{ch}", name=_nm(f"h1f{ch}")) for ch in range(CH)]
                if p2
                else None
            )

            # ---------------- emitters ----------------
            def emit_ff1(l, chs, inject=None):
                """ff1 pass. With two c-halves the chains are interleaved so
                consecutive matmuls share the stationary operand (one weight
                load serves both)."""
                for fti in range(FT // 2):
                    w1bt = w1pool.tile([P, 2, kb1, P], BF16, tag="w1b", name=_nm("w1b"))
                    nc.sync.dma_start(out=w1bt, in_=w1b_d[l, fti])
                    if p1:
                        w1ft = w1pool.tile([P, 2, 2 * p1, P], FP8, tag="w1f", name=_nm("w1f"))
                        nc.sync.dma_start(out=w1ft, in_=w1f_d[l, fti])
                    for fi in range(2):
                        ft = 2 * fti + fi
                        pts = {
                            ch: ps1.tile([P, CW], F32, tag="ps1", name=_nm("ps1"))
                            for ch in chs
                        }
                        for kt in range(kb1):
                            for ch in chs:
                                nc.tensor.matmul(
                                    pts[ch],
                                    lhsT=w1bt[:, fi, kt, :],
                                    rhs=htb[(ch, kt)][:, :],
                                    start=(kt == 0),
                                    stop=(kt == kb1 - 1 and not p1),
                                )
                        if p1:
                            for ch in chs:
                                nc.tensor.matmul(
                                    pts[ch],
                                    lhsT=w1ft[:, fi, :, :],
                                    rhs=htf[ch][:, :, :],
                                    start=False,
                                    stop=True,
                                    perf_mode=DR,
                                )
                        for ch in chs:
                            out_ap = (
                                h1b[ch][:, ft, :]
                                if ft < kb2
                                else h1f[ch][:, ft - kb2, :]
                            )
                            nc.scalar.activation(
                                out=out_ap,
                                in_=pts[ch],
                                func=mybir.ActivationFunctionType.Relu,
                                bias=b1_sb[:, l, ft : ft + 1],
                                scale=SIG1,
                            )
                    if inject and fti in inject:
                        for fn in inject[fti]:
                            fn()

            sqs = {}
            xbs = {}

            def emit_stats_mm(l, ch, dt, smu_t, sm2_t):
                nc.tensor.matmul(
                    smu_t,
                    lhsT=onesb[:, :],
                    rhs=xbs[(dt, ch)][:, :],
                    start=(dt == 0),
                    stop=(dt == DT - 1),
                )
                nc.tensor.matmul(
                    sm2_t,
                    lhsT=onesb[:, :],
                    rhs=sqs[(dt, ch)][:, :],
                    start=(dt == 0),
                    stop=(dt == DT - 1),
                )

            def emit_ff2(l, ch, inject=None):
                """ff2 pass over dt; returns carry closure (last stats MM)."""
                last = l == L - 1
                smu_t = sm2_t = None
                if not last:
                    smu_t = psm.tile([1, CW], F32, tag="smu", name=_nm("smu"))
                    sm2_t = psm.tile([1, CW], F32, tag="sm2", name=_nm("sm2"))
                carry = None
                for dt in range(DT):
                    w2bt = w2pool.tile([P, kb2, P], BF16, tag="w2b", name=_nm("w2b"))
                    nc.gpsimd.dma_start(out=w2bt, in_=w2b_d[l, dt])
                    if p2:
                        w2ft = w2pool.tile([P, p2, 2, P], FP8, tag="w2f", name=_nm("w2f"))
                        nc.gpsimd.dma_start(out=w2ft, in_=w2f_d[l, dt])
                    pt = ps2.tile([P, CW], F32, tag="ps2", name=_nm("ps2"))
                    for kt in range(kb2):
                        nc.tensor.matmul(
                            pt,
                            lhsT=w2bt[:, kt, :],
                            rhs=h1b[ch][:, kt, :],
                            start=(kt == 0),
                            stop=(kt == kb2 - 1 and not p2),
                        )
                    for a in range(p2):
                        nc.tensor.matmul(
                            pt,
                            lhsT=w2ft[:, a, :, :],
                            rhs=h1f[ch][:, 2 * a : 2 * a + 2, :],
                            start=False,
                            stop=(a == p2 - 1),
                            perf_mode=DR,
                        )
                    x = xs[(dt, ch)]
                    if with_b2:
                        tmp = tmps.tile([P, CW], F32, tag="evt", name=_nm("evt"))
                        nc.scalar.activation(
                            out=tmp,
                            in_=pt,
                            func=mybir.ActivationFunctionType.Identity,
                            bias=b2_sb[:, l, dt : dt + 1],
                            scale=1.0,
                        )
                        nc.vector.tensor_add(out=x, in0=x, in1=tmp)
                    else:
                        nc.vector.tensor_add(out=x, in0=x, in1=pt)
                    if last:
                        nc.vector.tensor_mul(
                            out=x, in0=x, in1=alpha_sb[:, ch * CW : (ch + 1) * CW]
                        )
                        nc.scalar.dma_start(
                            out=y_d[dt * P : (dt + 1) * P, ch * CW : (ch + 1) * CW],
                            in_=x,
                        )
                    else:
                        xb = tmps.tile([P, CW], BF16, tag="xb", name=_nm("xb"))
                        nc.vector.tensor_copy(out=xb, in_=x)
                        xbs[(dt, ch)] = xb
                        sq = tmps.tile([P, CW], BF16, tag="sq", name=_nm("sq"))
                        nc.vector.tensor_mul(out=sq, in0=xb, in1=xb)
                        sqs[(dt, ch)] = sq
                        # delay-slot: stats MM for dt-1 lands after chain dt
                        if dt > 0:
                            emit_stats_mm(l, ch, dt - 1, smu_t, sm2_t)
                    if inject and dt in inject:
                        for fn in inject[dt]:
                            fn()
                if not last:
                    carry = lambda: emit_stats_mm(l, ch, DT - 1, smu_t, sm2_t)
                return carry, smu_t, sm2_t

            def emit_ff2_pair(l):
                """Last-layer ff2: both c-halves interleaved per dt, sharing
                one w2 load; evict + gate + output DMA per (dt, ch)."""
                for dt in range(DT):
                    w2bt = w2pool.tile([P, kb2, P], BF16, tag="w2b", name=_nm("w2b"))
                    nc.gpsimd.dma_start(out=w2bt, in_=w2b_d[l, dt])
                    if p2:
                        w2ft = w2pool.tile([P, p2, 2, P], FP8, tag="w2f", name=_nm("w2f"))
                        nc.gpsimd.dma_start(out=w2ft, in_=w2f_d[l, dt])
                    pts = {
                        ch: ps2.tile([P, CW], F32, tag="ps2", name=_nm("ps2"))
                        for ch in range(CH)
                    }
                    for kt in range(kb2):
                        for ch in range(CH):
                            nc.tensor.matmul(
                                pts[ch],
                                lhsT=w2bt[:, kt, :],
                                rhs=h1b[ch][:, kt, :],
                                start=(kt == 0),
                                stop=(kt == kb2 - 1 and not p2),
                            )
                    for a in range(p2):
                        for ch in range(CH):
                            nc.tensor.matmul(
                                pts[ch],
                                lhsT=w2ft[:, a, :, :],
                                rhs=h1f[ch][:, 2 * a : 2 * a + 2, :],
                                start=False,
                                stop=(a == p2 - 1),
                                perf_mode=DR,
                            )
                    for ch in range(CH):
                        x = xs[(dt, ch)]
                        if with_b2:
                            tmp = tmps.tile([P, CW], F32, tag="evt", name=_nm("evt"))
                            nc.scalar.activation(
                                out=tmp,
                                in_=pts[ch],
                                func=mybir.ActivationFunctionType.Identity,
                                bias=b2_sb[:, l, dt : dt + 1],
                                scale=1.0,
                            )
                            nc.vector.tensor_add(out=x, in0=x, in1=tmp)
                        else:
                            nc.vector.tensor_add(out=x, in0=x, in1=pts[ch])
                        nc.vector.tensor_mul(
                            out=x, in0=x, in1=alpha_sb[:, ch * CW : (ch + 1) * CW]
                        )
                        nc.scalar.dma_start(
                            out=y_d[dt * P : (dt + 1) * P, ch * CW : (ch + 1) * CW],
                            in_=x,
                        )

            ab_sb = {}

            def make_finalize(l, ch, smu_t, sm2_t):
                def fin():
                    mu_sb = stat.tile([1, CW], F32, tag="mu_sb", name=_nm("mu_sb"))
                    nc.vector.tensor_copy(out=mu_sb, in_=smu_t)
                    musq = stat.tile([1, CW], F32, tag="musq", name=_nm("musq"))
                    nc.vector.tensor_mul(out=musq, in0=mu_sb, in1=mu_sb)
                    var = stat.tile([1, CW], F32, tag="var", name=_nm("var"))
                    nc.vector.tensor_sub(out=var, in0=sm2_t, in1=musq)
                    sd = stat.tile([1, CW], F32, tag="sd", name=_nm("sd"))
                    nc.scalar.activation(
                        out=sd,
                        in_=var,
                        func=mybir.ActivationFunctionType.Sqrt,
                        bias=eps_t,
                        scale=1.0 / (S_H * S_H),
                    )
                    A = stat.tile([1, CW], BF16, tag="A", name=_nm("A"))
                    muA = stat.tile([1, CW], BF16, tag="muA", name=_nm("muA"))
                    with nc.allow_low_precision(reason="rstd/mu*rstd in bf16 feed bf16 matmuls anyway"):
                        nc.vector.reciprocal(out=A, in_=sd)
                        nc.vector.tensor_mul(out=muA, in0=mu_sb, in1=A)
                    ab_sb[("A", ch)] = A
                    ab_sb[("muA", ch)] = muA

                return fin

            def make_bcast(l, ch):
                def bc():
                    for name in ("A", "muA"):
                        bct = psm.tile([P, CW], F32, tag="bc", name=_nm("bc"))
                        nc.tensor.matmul(
                            bct,
                            lhsT=ones1[:, :],
                            rhs=ab_sb[(name, ch)][:, :],
                            start=True,
                            stop=True,
                        )
                        sb = abpool.tile([P, CW], F32, tag=f"{name}b{ch}", name=_nm(f"{name}b{ch}"))
                        nc.vector.tensor_copy(out=sb, in_=bct)
                        ab_sb[(name + "b", ch)] = sb

                return bc

            def make_apply(l, ch):
                def ap():
                    Ab = ab_sb[("Ab", ch)]
                    mAb = ab_sb[("muAb", ch)]
                    for dt in range(DT):
                        tmp = tmps.tile([P, CW], F32, tag="tap", name=_nm("tap"))
                        nc.vector.tensor_mul(out=tmp, in0=xs[(dt, ch)], in1=Ab)
                        out_ap = (
                            htb[(ch, dt)][:, :]
                            if dt < kb1
                            else htf[ch][:, dt - kb1, :]
                        )
                        nc.vector.tensor_sub(out=out_ap, in0=tmp, in1=mAb)

                return ap

            # ---------------- program ----------------
            emit_ff1(0, [0, 1])
            carry0, smu0, sm20 = emit_ff2(0, 0)
            fin0 = make_finalize(0, 0, smu0, sm20)
            carry1, smu1, sm21 = emit_ff2(
                0,
                1,
                inject={
                    0: [carry0],
                    1: [fin0],
                    2: [make_bcast(0, 0)],
                    3: [make_apply(0, 0)],
                },
            )
            fin1 = make_finalize(0, 1, smu1, sm21)
            emit_ff1(
                1,
                [0],
                inject={
                    0: [carry1],
                    1: [fin1],
                    2: [make_bcast(0, 1)],
                    3: [make_apply(0, 1)],
                },
            )
            emit_ff1(1, [1])
            nc.sync.dma_start(out=alpha_sb, in_=alpha_d[:, :])
            emit_ff2(1, 0)
            emit_ff2(1, 1)

    if split_waits:
        _split_multi_waits(nc)
    return nc


_NC_CACHE = {}


def _get_nc(with_b2):
    key = (P1, P2, with_b2)
    if key not in _NC_CACHE:
        _NC_CACHE[key] = build_bass(P1, P2, with_b2)
    return _NC_CACHE[key]


# ---------------------------------------------------------------------------
# Host side
# ---------------------------------------------------------------------------
def _routing_perm(features, centroids):
    """Replicates reference._balanced_assignment bit-for-bit on CPU jax."""
    import jax
    import jax.numpy as jnp

    with jax.default_device(jax.devices("cpu")[0]):
        feats = jnp.asarray(features)
        cents = jnp.asarray(centroids)
        aff = jax.lax.stop_gradient(feats) @ jax.lax.stop_gradient(cents).T
        aff = jnp.nan_to_num(aff)
        capacity = feats.shape[0] // cents.shape[0]
        order = jnp.argsort(-aff.max(axis=1))
        aff_ord = aff[order]

        def step(counts, row):
            masked = jnp.where(counts < capacity, row, -jnp.inf)
            e = jnp.argmax(masked).astype(jnp.int32)
            return counts.at[e].add(1), e

        _, assign_ord = jax.lax.scan(
            step, jnp.zeros(cents.shape[0], jnp.int32), aff_ord
        )
        assign = jnp.zeros(feats.shape[0], jnp.int32).at[order].set(assign_ord)
        return np.asarray(jnp.argsort(assign))


def _q8(x, scale):
    import ml_dtypes

    return np.clip(x * scale, -240.0, 240.0).astype(ml_dtypes.float8_e4m3)


def _swi(wpair):
    """[..., 2, M] pair -> DoubleRowSwInterleave layout [..., 2, M]:
    stream A[M-1], B[M-1], A[M-2], ... per partition row."""
    a = wpair[..., 0, ::-1]
    b = wpair[..., 1, ::-1]
    out = np.empty_like(wpair).reshape(*wpair.shape[:-2], 2 * wpair.shape[-1])
    out[..., 0::2] = a
    out[..., 1::2] = b
    return out.reshape(wpair.shape)


def _prep_core_inputs(xr, centroids, ln_gamma, ln_beta, W1, b1, W2, b2):
    import ml_dtypes

    kb1 = DT - 2 * P1
    kb2 = FT - 2 * P2
    with_b2 = bool(np.any(b2))
    maps = []
    for e in range(E):
        x = xr[e].astype(np.float32)  # [C, D]
        m = {}
        # alpha gate (computed from unscaled x), shipped /kappa, broadcast
        aff = x @ centroids[e].astype(np.float32)
        alpha = 1.0 / (1.0 + np.exp(-aff)) / KAPPA
        m["alpha"] = np.ascontiguousarray(
            np.broadcast_to(alpha[None, :].astype(np.float32), (P, C))
        )
        # layer-0 LN on host
        mu = x.mean(-1, keepdims=True)
        var = ((x - mu) ** 2).mean(-1, keepdims=True)
        h0 = (x - mu) / np.sqrt(var + EPS)  # [C, D]
        h0t = np.ascontiguousarray(h0.T * S_H).reshape(DT, P, C)  # [dt, p, c]
        m["ht0b"] = np.ascontiguousarray(
            h0t[:kb1].transpose(1, 0, 2)
        ).astype(ml_dtypes.bfloat16)
        if P1:
            m["ht0f"] = np.ascontiguousarray(
                np.clip(h0t[kb1:], -240.0, 240.0).transpose(1, 0, 2)
            ).astype(ml_dtypes.float8_e4m3)
        m["x"] = np.ascontiguousarray(x.T * KAPPA).astype(np.float32)

        w1b = np.empty((L, FT // 2, P, 2, kb1, P), ml_dtypes.bfloat16)
        w1f = np.empty((L, FT // 2, P, 2, 2 * P1, P), ml_dtypes.float8_e4m3)
        w2b = np.empty((L, DT, P, kb2, P), ml_dtypes.bfloat16)
        w2f = np.empty((L, DT, P, P2, 2, P), ml_dtypes.float8_e4m3)
        b1s = np.empty((L, P, FT), np.float32)
        b2s = np.empty((L, P, DT), np.float32)
        for l in range(L):
            g = ln_gamma[l, e].astype(np.float32)
            bt = ln_beta[l, e].astype(np.float32)
            w1_eff = W1[l, e].astype(np.float32) * g[None, :]  # [F, D]
            b1_eff = (b1[l, e] + W1[l, e] @ bt).astype(np.float32) * S_H1
            # [fti, fi, m, kt, p] -> [fti, p, fi, kt, m]
            a = w1_eff.reshape(FT // 2, 2, P, DT, P).transpose(0, 4, 1, 3, 2)
            w1b[l] = (a[:, :, :, :kb1, :] * S_W1).astype(ml_dtypes.bfloat16)
            if P1:
                w1f[l] = _swi(_q8(a[:, :, :, kb1:, :], S_W1))
            # W2 [D, F] -> [dt, m, kt, p] -> [dt, p, kt, m]
            a2 = W2[l, e].astype(np.float32).reshape(DT, P, FT, P).transpose(
                0, 3, 2, 1
            )
            w2b[l] = (a2[:, :, :kb2, :] * S_W2).astype(ml_dtypes.bfloat16)
            if P2:
                w2f[l] = _swi(
                    _q8(a2[:, :, kb2:, :].reshape(DT, P, P2, 2, P), S_W2)
                )
            b1s[l] = b1_eff.reshape(FT, P).T
            b2s[l] = (b2[l, e].astype(np.float32) * KAPPA).reshape(DT, P).T
        m["w1b"] = w1b
        m["w2b"] = w2b
        if P1:
            m["w1f"] = w1f
        if P2:
            m["w2f"] = w2f
        m["b1"] = b1s
        if with_b2:
            m["b2"] = b2s
        maps.append(m)
    return maps, with_b2


def kernel(
    input_features,
    centroids,
    ln_gamma,
    ln_beta,
    W1,
    b1,
    W2,
    b2,
    input_ids=None,
    _trace=False,
    _tmpdir=None,
):
    input_features = np.asarray(input_features, np.float32)
    centroids = np.asarray(centroids, np.float32)
    ln_gamma = np.asarray(ln_gamma, np.float32)
    ln_beta = np.asarray(ln_beta, np.float32)
    W1 = np.asarray(W1, np.float32)
    b1 = np.asarray(b1, np.float32)
    W2 = np.asarray(W2, np.float32)
    b2 = np.asarray(b2, np.float32)

    feats = input_features.reshape(T, D)
    perm = _routing_perm(feats, centroids)
    xr = feats[perm].reshape(E, C, D)

    maps, with_b2 = _prep_core_inputs(
        xr, centroids, ln_gamma, ln_beta, W1, b1, W2, b2
    )
    nc = _get_nc(with_b2)
    res = run_bass_kernel_spmd(
        nc, maps, list(range(E)), trace=_trace, tmpdir=_tmpdir
    )
    y = np.concatenate(
        [np.ascontiguousarray(res.results[e]["y"].T) for e in range(E)], axis=0
    )
    out = np.zeros((T, D), np.float32)
    out[perm] = y
    out = out.reshape(input_features.shape)
    if _trace:
        return out, res
    return out


# revision 8
# speedup vs baseline: 1.0042x; 1.0006x over previous
"""BASE-layer MoE kernel v2 for Trainium2, expert-parallel across 8 NeuronCores.

Changes vs v1 (504 us):
  - Residual stream kept transposed [d, c] end-to-end: host ships x^T and the
    layer-0 LN output h0^T pre-tiled, ff2 is computed in [d, c] orientation,
    so the 128 PE transposes (~35 us) disappear. Output y^T is transposed
    back on the host.
  - LN stats for layer 1 are computed on PE with ones-matmuls over the
    residual (fp32r moving operand = 1 cycle/row), finalized on [1, C]
    vectors, broadcast back to [128, C] via K=1 matmuls.
  - A tuned fraction of the contraction runs in fp8 e4m3 with
    perf_mode=DoubleRow (2 K-tiles per matmul): the last 2*P1 of 8 d-tiles in
    ff1 and the last 2*P2 of 32 f-tiles in ff2. Config (P1=1, P2=2) measures
    1.77e-2 rel err in simulation vs the 2e-2 budget (bf16-only: 2.5e-3).
  - Everything lives in a power-of-2 scaled domain so bf16/fp8 parts share
    one PSUM accumulation chain: residual x~ = kappa*x with
    kappa = S_H1*S_W2; LN is scale-invariant once eps is scaled by kappa^2.
"""

import contextlib

import numpy as np

import concourse.bass as bass
import concourse.mybir as mybir
import concourse.tile as tile
from concourse.bass_utils import run_bass_kernel_spmd

S, B, D, F, E, L = 2048, 4, 1024, 4096, 8, 2
EPS = 1e-5
T = S * B
C = T // E
P = 128
CH, CW = 2, 512          # c halves
DT = D // P              # 8 d tiles
FT = F // P              # 32 f tiles

# fp8 DoubleRow config: last 2*P1 d-tiles of ff1 / last 2*P2 f-tiles of ff2
P1, P2 = 1, 2
S_H, S_W1, S_H1, S_W2 = 16.0, 64.0, 32.0, 32.0
KAPPA = S_H1 * S_W2                      # residual scale (1024, power of 2)
SIG1 = S_H1 / (S_H * S_W1)               # ff1 psum -> h1 rescale
EPS_SQ = EPS * KAPPA * KAPPA / (S_H * S_H)   # bias for the sqrt in 1/A
RD = 1.0 / D

F32 = mybir.dt.float32
F32R = mybir.dt.float32r
BF16 = mybir.dt.bfloat16
FP8 = mybir.dt.float8e4
DR = mybir.MatmulPerfMode.DoubleRowSwInterleave

_MAX_WAITS = 1
_NAME_CTR = [0]


def _nm(base):
    _NAME_CTR[0] += 1
    return f"{base}_{_NAME_CTR[0]}"


def _split_multi_waits(nc, limit=_MAX_WAITS):
    """walrus build rejects >1 sync wait per instruction; split onto NOPs."""
    n_split = 0
    for f in nc.m.functions:
        for bb in f.blocks:
            out = []
            changed = False
            for ins in bb.instructions:
                si = getattr(ins, "sync_info", None)
                if si is not None and si.on_wait and len(si.on_wait) > limit:
                    waits = list(si.on_wait)
                    head, tail = waits[:-limit], waits[-limit:]
                    for i in range(0, len(head), limit):
                        n_split += 1
                        nop = mybir.InstNoOp(
                            name=f"waitsplit-{n_split}",
                            engine=ins.engine,
                            text_hint="waitsplit",
                            bass_nofuse=True,
                        )
                        nop.sync_info = mybir.SyncInfo(
                            on_wait=head[i : i + limit], on_update=[]
                        )
                        out.append(nop)
                    ins.sync_info = mybir.SyncInfo(
                        on_wait=tail, on_update=list(si.on_update or [])
                    )
                    changed = True
                out.append(ins)
            if changed:
                bb.instructions = out
    return n_split


def build_bass(p1=P1, p2=P2, with_b2=False, split_waits=True):
    kb1 = DT - 2 * p1
    kb2 = FT - 2 * p2
    nc = bass.Bass()
    x_d = nc.declare_dram_parameter("x", [D, C], F32, isOutput=False)
    ht0b_d = nc.declare_dram_parameter("ht0b", [P, kb1, C], BF16, isOutput=False)
    if p1:
        ht0f_d = nc.declare_dram_parameter("ht0f", [P, 2 * p1, C], FP8, isOutput=False)
    w1b_d = nc.declare_dram_parameter(
        "w1b", [L, FT // 2, P, 2, kb1, P], BF16, isOutput=False
    )
    if p1:
        w1f_d = nc.declare_dram_parameter(
            "w1f", [L, FT // 2, P, 2, 2 * p1, P], FP8, isOutput=False
        )
    w2b_d = nc.declare_dram_parameter("w2b", [L, DT, P, kb2, P], BF16, isOutput=False)
    if p2:
        w2f_d = nc.declare_dram_parameter(
            "w2f", [L, DT, P, p2, 2, P], FP8, isOutput=False
        )
    b1_d = nc.declare_dram_parameter("b1", [L, P, FT], F32, isOutput=False)
    alpha_d = nc.declare_dram_parameter("alpha", [P, C], F32, isOutput=False)
    if with_b2:
        b2_d = nc.declare_dram_parameter("b2", [L, P, DT], F32, isOutput=False)
    y_d = nc.declare_dram_parameter("y", [D, C], F32, isOutput=True)

    with tile.TileContext(nc) as tc:
        with contextlib.ExitStack() as ctx:
            singles = ctx.enter_context(tc.tile_pool(name="singles", bufs=1))
            xspool = ctx.enter_context(tc.tile_pool(name="xspool", bufs=1))
            htpool = ctx.enter_context(tc.tile_pool(name="htpool", bufs=1))
            h1pool = ctx.enter_context(tc.tile_pool(name="h1pool", bufs=1))
            w1pool = ctx.enter_context(tc.tile_pool(name="w1pool", bufs=8))
            w2pool = ctx.enter_context(tc.tile_pool(name="w2pool", bufs=3))
            stat = ctx.enter_context(tc.tile_pool(name="stat", bufs=2))
            tmps = ctx.enter_context(tc.tile_pool(name="tmps", bufs=3))
            abpool = ctx.enter_context(tc.tile_pool(name="abpool", bufs=1))
            ps1 = ctx.enter_context(tc.tile_pool(name="ps1", bufs=3, space="PSUM"))
            ps2 = ctx.enter_context(tc.tile_pool(name="ps2", bufs=2, space="PSUM"))
            psm = ctx.enter_context(tc.tile_pool(name="psm", bufs=1, space="PSUM"))

            # ---- layer-0 h^T (host-precomputed LN output), ch0 first ----
            # per-(ch, kt) tiles: fine DMA granularity so the first ff1 chain
            # starts as soon as the first 256 KB lands
            htb = {
                (ch, kt): htpool.tile(
                    [P, CW], BF16, tag=f"htb{ch}_{kt}", name=_nm(f"htb{ch}_{kt}")
                )
                for ch in range(CH)
                for kt in range(kb1)
            }
            htf = (
                [htpool.tile([P, 2 * p1, CW], FP8, tag=f"htf{ch}", name=_nm(f"htf{ch}")) for ch in range(CH)]
                if p1
                else None
            )
            for ch in range(CH):
                csl = slice(ch * CW, (ch + 1) * CW)
                for kt in range(kb1):
                    eng = nc.gpsimd if kt % 2 == 0 else nc.scalar
                    eng.dma_start(out=htb[(ch, kt)], in_=ht0b_d[:, kt, csl])
                if p1:
                    nc.gpsimd.dma_start(out=htf[ch], in_=ht0f_d[:, :, csl])

            # ---- small consts + residual x~ ----
            b1_sb = singles.tile([P, L, FT], F32)
            for l in range(L):
                nc.scalar.dma_start(out=b1_sb[:, l, :], in_=b1_d[l])
            if with_b2:
                b2_sb = singles.tile([P, L, DT], F32)
                for l in range(L):
                    nc.scalar.dma_start(out=b2_sb[:, l, :], in_=b2_d[l])
            xs = {}
            for dt in range(DT):
                for ch in range(CH):
                    t = xspool.tile([P, CW], F32, tag=f"xs{dt}_{ch}", name=_nm(f"xs{dt}_{ch}"))
                    nc.gpsimd.dma_start(
                        out=t,
                        in_=x_d[dt * P : (dt + 1) * P, ch * CW : (ch + 1) * CW],
                    )
                    xs[(dt, ch)] = t
            alpha_sb = singles.tile([P, C], F32)

            onesb = singles.tile([P, 1], BF16)
            nc.gpsimd.memset(onesb, RD)
            ones1 = singles.tile([1, P], BF16)
            nc.gpsimd.memset(ones1, 1.0)
            eps_t = singles.tile([1, 1], F32)
            nc.gpsimd.memset(eps_t, EPS_SQ)

            h1b = [h1pool.tile([P, kb2, CW], BF16, tag=f"h1b{ch}", name=_nm(f"h1b{ch}")) for ch in range(CH)]
            h1f = (
                [h1pool.tile([P, 2 * p2, CW], FP8, tag=f"h1f{ch}", name=_nm(f"h1f{ch}")) for ch in range(CH)]
                if p2
                else None
            )

            # ---------------- emitters ----------------
            def emit_ff1(l, chs, inject=None):
                """ff1 pass. With two c-halves the chains are interleaved so
                consecutive matmuls share the stationary operand (one weight
                load serves both)."""
                for fti in range(FT // 2):
                    w1bt = w1pool.tile([P, 2, kb1, P], BF16, tag="w1b", name=_nm("w1b"))
                    nc.sync.dma_start(out=w1bt, in_=w1b_d[l, fti])
                    if p1:
                        w1ft = w1pool.tile([P, 2, 2 * p1, P], FP8, tag="w1f", name=_nm("w1f"))
                        nc.sync.dma_start(out=w1ft, in_=w1f_d[l, fti])
                    for fi in range(2):
                        ft = 2 * fti + fi
                        pts = {
                            ch: ps1.tile([P, CW], F32, tag="ps1", name=_nm("ps1"))
                            for ch in chs
                        }
                        # early chunks trickle in ch0-first: keep the first
                        # chains single-channel so they never wait on ch1
                        seq = l == 0 and fti < 2
                        kt_ch = (
                            [(kt, ch) for ch in chs for kt in range(kb1)]
                            if seq
                            else [(kt, ch) for kt in range(kb1) for ch in chs]
                        )
                        for kt, ch in kt_ch:
                            if True:
                                nc.tensor.matmul(
                                    pts[ch],
                                    lhsT=w1bt[:, fi, kt, :],
                                    rhs=htb[(ch, kt)][:, :],
                                    start=(kt == 0),
                                    stop=(kt == kb1 - 1 and not p1),
                                )
                        if p1:
                            for ch in chs:
                                nc.tensor.matmul(
                                    pts[ch],
                                    lhsT=w1ft[:, fi, :, :],
                                    rhs=htf[ch][:, :, :],
                                    start=False,
                                    stop=True,
                                    perf_mode=DR,
                                )
                        for ch in chs:
                            out_ap = (
                                h1b[ch][:, ft, :]
                                if ft < kb2
                                else h1f[ch][:, ft - kb2, :]
                            )
                            nc.scalar.activation(
                                out=out_ap,
                                in_=pts[ch],
                                func=mybir.ActivationFunctionType.Relu,
                                bias=b1_sb[:, l, ft : ft + 1],
                                scale=SIG1,
                            )
                    if inject and fti in inject:
                        for fn in inject[fti]:
                            fn()

            sqs = {}
            xbs = {}

            def emit_stats_mm(l, ch, dt, smu_t, sm2_t):
                nc.tensor.matmul(
                    smu_t,
                    lhsT=onesb[:, :],
                    rhs=xbs[(dt, ch)][:, :],
                    start=(dt == 0),
                    stop=(dt == DT - 1),
                )
                nc.tensor.matmul(
                    sm2_t,
                    lhsT=onesb[:, :],
                    rhs=sqs[(dt, ch)][:, :],
                    start=(dt == 0),
                    stop=(dt == DT - 1),
                )

            def emit_ff2(l, ch, inject=None):
                """ff2 pass over dt; returns carry closure (last stats MM)."""
                last = l == L - 1
                smu_t = sm2_t = None
                if not last:
                    smu_t = psm.tile([1, CW], F32, tag="smu", name=_nm("smu"))
                    sm2_t = psm.tile([1, CW], F32, tag="sm2", name=_nm("sm2"))
                carry = None
                for dt in range(DT):
                    w2bt = w2pool.tile([P, kb2, P], BF16, tag="w2b", name=_nm("w2b"))
                    nc.gpsimd.dma_start(out=w2bt, in_=w2b_d[l, dt])
                    if p2:
                        w2ft = w2pool.tile([P, p2, 2, P], FP8, tag="w2f", name=_nm("w2f"))
                        nc.gpsimd.dma_start(out=w2ft, in_=w2f_d[l, dt])
                    pt = ps2.tile([P, CW], F32, tag="ps2", name=_nm("ps2"))
                    for kt in range(kb2):
                        nc.tensor.matmul(
                            pt,
                            lhsT=w2bt[:, kt, :],
                            rhs=h1b[ch][:, kt, :],
                            start=(kt == 0),
                            stop=(kt == kb2 - 1 and not p2),
                        )
                    for a in range(p2):
                        nc.tensor.matmul(
                            pt,
                            lhsT=w2ft[:, a, :, :],
                            rhs=h1f[ch][:, 2 * a : 2 * a + 2, :],
                            start=False,
                            stop=(a == p2 - 1),
                            perf_mode=DR,
                        )
                    x = xs[(dt, ch)]
                    if with_b2:
                        tmp = tmps.tile([P, CW], F32, tag="evt", name=_nm("evt"))
                        nc.scalar.activation(
                            out=tmp,
                            in_=pt,
                            func=mybir.ActivationFunctionType.Identity,
                            bias=b2_sb[:, l, dt : dt + 1],
                            scale=1.0,
                        )
                        nc.vector.tensor_add(out=x, in0=x, in1=tmp)
                    else:
                        nc.vector.tensor_add(out=x, in0=x, in1=pt)
                    if last:
                        nc.vector.tensor_mul(
                            out=x, in0=x, in1=alpha_sb[:, ch * CW : (ch + 1) * CW]
                        )
                        nc.scalar.dma_start(
                            out=y_d[dt * P : (dt + 1) * P, ch * CW : (ch + 1) * CW],
                            in_=x,
                        )
                    else:
                        xb = tmps.tile([P, CW], BF16, tag="xb", name=_nm("xb"))
                        nc.vector.tensor_copy(out=xb, in_=x)
                        xbs[(dt, ch)] = xb
                        sq = tmps.tile([P, CW], BF16, tag="sq", name=_nm("sq"))
                        nc.vector.tensor_mul(out=sq, in0=xb, in1=xb)
                        sqs[(dt, ch)] = sq
                        # delay-slot: stats MM for dt-1 lands after chain dt
                        if dt > 0:
                            emit_stats_mm(l, ch, dt - 1, smu_t, sm2_t)
                    if inject and dt in inject:
                        for fn in inject[dt]:
                            fn()
                if not last:
                    carry = lambda: emit_stats_mm(l, ch, DT - 1, smu_t, sm2_t)
                return carry, smu_t, sm2_t

            def emit_ff2_pair(l):
                """Last-layer ff2: both c-halves interleaved per dt, sharing
                one w2 load; evict + gate + output DMA per (dt, ch)."""
                for dt in range(DT):
                    w2bt = w2pool.tile([P, kb2, P], BF16, tag="w2b", name=_nm("w2b"))
                    nc.gpsimd.dma_start(out=w2bt, in_=w2b_d[l, dt])
                    if p2:
                        w2ft = w2pool.tile([P, p2, 2, P], FP8, tag="w2f", name=_nm("w2f"))
                        nc.gpsimd.dma_start(out=w2ft, in_=w2f_d[l, dt])
                    pts = {
                        ch: ps2.tile([P, CW], F32, tag="ps2", name=_nm("ps2"))
                        for ch in range(CH)
                    }
                    for kt in range(kb2):
                        for ch in range(CH):
                            nc.tensor.matmul(
                                pts[ch],
                                lhsT=w2bt[:, kt, :],
                                rhs=h1b[ch][:, kt, :],
                                start=(kt == 0),
                                stop=(kt == kb2 - 1 and not p2),
                            )
                    for a in range(p2):
                        for ch in range(CH):
                            nc.tensor.matmul(
                                pts[ch],
                                lhsT=w2ft[:, a, :, :],
                                rhs=h1f[ch][:, 2 * a : 2 * a + 2, :],
                                start=False,
                                stop=(a == p2 - 1),
                                perf_mode=DR,
                            )
                    for ch in range(CH):
                        x = xs[(dt, ch)]
                        if with_b2:
                            tmp = tmps.tile([P, CW], F32, tag="evt", name=_nm("evt"))
                            nc.scalar.activation(
                                out=tmp,
                                in_=pts[ch],
                                func=mybir.ActivationFunctionType.Identity,
                                bias=b2_sb[:, l, dt : dt + 1],
                                scale=1.0,
                            )
                            nc.vector.tensor_add(out=x, in0=x, in1=tmp)
                        else:
                            nc.vector.tensor_add(out=x, in0=x, in1=pts[ch])
                        nc.vector.tensor_mul(
                            out=x, in0=x, in1=alpha_sb[:, ch * CW : (ch + 1) * CW]
                        )
                        nc.scalar.dma_start(
                            out=y_d[dt * P : (dt + 1) * P, ch * CW : (ch + 1) * CW],
                            in_=x,
                        )

            ab_sb = {}

            def make_finalize(l, ch, smu_t, sm2_t):
                def fin():
                    mu_sb = stat.tile([1, CW], F32, tag="mu_sb", name=_nm("mu_sb"))
                    nc.vector.tensor_copy(out=mu_sb, in_=smu_t)
                    musq = stat.tile([1, CW], F32, tag="musq", name=_nm("musq"))
                    nc.vector.tensor_mul(out=musq, in0=mu_sb, in1=mu_sb)
                    var = stat.tile([1, CW], F32, tag="var", name=_nm("var"))
                    nc.vector.tensor_sub(out=var, in0=sm2_t, in1=musq)
                    sd = stat.tile([1, CW], F32, tag="sd", name=_nm("sd"))
                    nc.scalar.activation(
                        out=sd,
                        in_=var,
                        func=mybir.ActivationFunctionType.Sqrt,
                        bias=eps_t,
                        scale=1.0 / (S_H * S_H),
                    )
                    A = stat.tile([1, CW], BF16, tag="A", name=_nm("A"))
                    muA = stat.tile([1, CW], BF16, tag="muA", name=_nm("muA"))
                    with nc.allow_low_precision(reason="rstd/mu*rstd in bf16 feed bf16 matmuls anyway"):
                        nc.vector.reciprocal(out=A, in_=sd)
                        nc.vector.tensor_mul(out=muA, in0=mu_sb, in1=A)
                    ab_sb[("A", ch)] = A
                    ab_sb[("muA", ch)] = muA

                return fin

            def make_bcast(l, ch):
                def bc():
                    for name in ("A", "muA"):
                        bct = psm.tile([P, CW], F32, tag="bc", name=_nm("bc"))
                        nc.tensor.matmul(
                            bct,
                            lhsT=ones1[:, :],
                            rhs=ab_sb[(name, ch)][:, :],
                            start=True,
                            stop=True,
                        )
                        sb = abpool.tile([P, CW], F32, tag=f"{name}b{ch}", name=_nm(f"{name}b{ch}"))
                        nc.vector.tensor_copy(out=sb, in_=bct)
                        ab_sb[(name + "b", ch)] = sb

                return bc

            def make_apply(l, ch):
                def ap():
                    Ab = ab_sb[("Ab", ch)]
                    mAb = ab_sb[("muAb", ch)]
                    for dt in range(DT):
                        tmp = tmps.tile([P, CW], F32, tag="tap", name=_nm("tap"))
                        nc.vector.tensor_mul(out=tmp, in0=xs[(dt, ch)], in1=Ab)
                        out_ap = (
                            htb[(ch, dt)][:, :]
                            if dt < kb1
                            else htf[ch][:, dt - kb1, :]
                        )
                        nc.vector.tensor_sub(out=out_ap, in0=tmp, in1=mAb)

                return ap

            # ---------------- program ----------------
            emit_ff1(0, [0, 1])
            carry0, smu0, sm20 = emit_ff2(0, 0)
            fin0 = make_finalize(0, 0, smu0, sm20)
            carry1, smu1, sm21 = emit_ff2(
                0,
                1,
                inject={
                    0: [carry0],
                    1: [fin0],
                    2: [make_bcast(0, 0)],
                    3: [make_apply(0, 0)],
                },
            )
            fin1 = make_finalize(0, 1, smu1, sm21)
            emit_ff1(
                1,
                [0],
                inject={
                    0: [carry1],
                    1: [fin1],
                    2: [make_bcast(0, 1)],
                    3: [make_apply(0, 1)],
                },
            )
            emit_ff1(1, [1])
            nc.sync.dma_start(out=alpha_sb, in_=alpha_d[:, :])
            emit_ff2(1, 0)
            emit_ff2(1, 1)

    if split_waits:
        _split_multi_waits(nc)
    return nc


_NC_CACHE = {}


def _get_nc(with_b2):
    key = (P1, P2, with_b2)
    if key not in _NC_CACHE:
        _NC_CACHE[key] = build_bass(P1, P2, with_b2)
    return _NC_CACHE[key]


# ---------------------------------------------------------------------------
# Host side
# ---------------------------------------------------------------------------
def _routing_perm(features, centroids):
    """Replicates reference._balanced_assignment bit-for-bit on CPU jax."""
    import jax
    import jax.numpy as jnp

    with jax.default_device(jax.devices("cpu")[0]):
        feats = jnp.asarray(features)
        cents = jnp.asarray(centroids)
        aff = jax.lax.stop_gradient(feats) @ jax.lax.stop_gradient(cents).T
        aff = jnp.nan_to_num(aff)
        capacity = feats.shape[0] // cents.shape[0]
        order = jnp.argsort(-aff.max(axis=1))
        aff_ord = aff[order]

        def step(counts, row):
            masked = jnp.where(counts < capacity, row, -jnp.inf)
            e = jnp.argmax(masked).astype(jnp.int32)
            return counts.at[e].add(1), e

        _, assign_ord = jax.lax.scan(
            step, jnp.zeros(cents.shape[0], jnp.int32), aff_ord
        )
        assign = jnp.zeros(feats.shape[0], jnp.int32).at[order].set(assign_ord)
        return np.asarray(jnp.argsort(assign))


def _q8(x, scale):
    import ml_dtypes

    return np.clip(x * scale, -240.0, 240.0).astype(ml_dtypes.float8_e4m3)


def _swi(wpair):
    """[..., 2, M] pair -> DoubleRowSwInterleave layout [..., 2, M]:
    stream A[M-1], B[M-1], A[M-2], ... per partition row."""
    a = wpair[..., 0, ::-1]
    b = wpair[..., 1, ::-1]
    out = np.empty_like(wpair).reshape(*wpair.shape[:-2], 2 * wpair.shape[-1])
    out[..., 0::2] = a
    out[..., 1::2] = b
    return out.reshape(wpair.shape)


def _prep_core_inputs(xr, centroids, ln_gamma, ln_beta, W1, b1, W2, b2):
    import ml_dtypes

    kb1 = DT - 2 * P1
    kb2 = FT - 2 * P2
    with_b2 = bool(np.any(b2))
    maps = []
    for e in range(E):
        x = xr[e].astype(np.float32)  # [C, D]
        m = {}
        # alpha gate (computed from unscaled x), shipped /kappa, broadcast
        aff = x @ centroids[e].astype(np.float32)
        alpha = 1.0 / (1.0 + np.exp(-aff)) / KAPPA
        m["alpha"] = np.ascontiguousarray(
            np.broadcast_to(alpha[None, :].astype(np.float32), (P, C))
        )
        # layer-0 LN on host
        mu = x.mean(-1, keepdims=True)
        var = ((x - mu) ** 2).mean(-1, keepdims=True)
        h0 = (x - mu) / np.sqrt(var + EPS)  # [C, D]
        h0t = np.ascontiguousarray(h0.T * S_H).reshape(DT, P, C)  # [dt, p, c]
        m["ht0b"] = np.ascontiguousarray(
            h0t[:kb1].transpose(1, 0, 2)
        ).astype(ml_dtypes.bfloat16)
        if P1:
            m["ht0f"] = np.ascontiguousarray(
                np.clip(h0t[kb1:], -240.0, 240.0).transpose(1, 0, 2)
            ).astype(ml_dtypes.float8_e4m3)
        m["x"] = np.ascontiguousarray(x.T * KAPPA).astype(np.float32)

        w1b = np.empty((L, FT // 2, P, 2, kb1, P), ml_dtypes.bfloat16)
        w1f = np.empty((L, FT // 2, P, 2, 2 * P1, P), ml_dtypes.float8_e4m3)
        w2b = np.empty((L, DT, P, kb2, P), ml_dtypes.bfloat16)
        w2f = np.empty((L, DT, P, P2, 2, P), ml_dtypes.float8_e4m3)
        b1s = np.empty((L, P, FT), np.float32)
        b2s = np.empty((L, P, DT), np.float32)
        for l in range(L):
            g = ln_gamma[l, e].astype(np.float32)
            bt = ln_beta[l, e].astype(np.float32)
            w1_eff = W1[l, e].astype(np.float32) * g[None, :]  # [F, D]
            b1_eff = (b1[l, e] + W1[l, e] @ bt).astype(np.float32) * S_H1
            # [fti, fi, m, kt, p] -> [fti, p, fi, kt, m]
            a = w1_eff.reshape(FT // 2, 2, P, DT, P).transpose(0, 4, 1, 3, 2)
            w1b[l] = (a[:, :, :, :kb1, :] * S_W1).astype(ml_dtypes.bfloat16)
            if P1:
                w1f[l] = _swi(_q8(a[:, :, :, kb1:, :], S_W1))
            # W2 [D, F] -> [dt, m, kt, p] -> [dt, p, kt, m]
            a2 = W2[l, e].astype(np.float32).reshape(DT, P, FT, P).transpose(
                0, 3, 2, 1
            )
            w2b[l] = (a2[:, :, :kb2, :] * S_W2).astype(ml_dtypes.bfloat16)
            if P2:
                w2f[l] = _swi(
                    _q8(a2[:, :, kb2:, :].reshape(DT, P, P2, 2, P), S_W2)
                )
            b1s[l] = b1_eff.reshape(FT, P).T
            b2s[l] = (b2[l, e].astype(np.float32) * KAPPA).reshape(DT, P).T
        m["w1b"] = w1b
        m["w2b"] = w2b
        if P1:
            m["w1f"] = w1f
        if P2:
            m["w2f"] = w2f
        m["b1"] = b1s
        if with_b2:
            m["b2"] = b2s
        maps.append(m)
    return maps, with_b2


def kernel(
    input_features,
    centroids,
    ln_gamma,
    ln_beta,
    W1,
    b1,
    W2,
    b2,
    input_ids=None,
    _trace=False,
    _tmpdir=None,
):
    input_features = np.asarray(input_features, np.float32)
    centroids = np.asarray(centroids, np.float32)
    ln_gamma = np.asarray(ln_gamma, np.float32)
    ln_beta = np.asarray(ln_beta, np.float32)
    W1 = np.asarray(W1, np.float32)
    b1 = np.asarray(b1, np.float32)
    W2 = np.asarray(W2, np.float32)
    b2 = np.asarray(b2, np.float32)

    feats = input_features.reshape(T, D)
    perm = _routing_perm(feats, centroids)
    xr = feats[perm].reshape(E, C, D)

    maps, with_b2 = _prep_core_inputs(
        xr, centroids, ln_gamma, ln_beta, W1, b1, W2, b2
    )
    nc = _get_nc(with_b2)
    res = run_bass_kernel_spmd(
        nc, maps, list(range(E)), trace=_trace, tmpdir=_tmpdir
    )
    y = np.concatenate(
        [np.ascontiguousarray(res.results[e]["y"].T) for e in range(E)], axis=0
    )
    out = np.zeros((T, D), np.float32)
    out[perm] = y
    out = out.reshape(input_features.shape)
    if _trace:
        return out, res
    return out
